# revision 16
# baseline (speedup 1.0000x reference)
"""Binarized BasicBlock (BNN) forward on 8 Trainium2 NeuronCores.

Reference computation (per reference.py):
    xb  = sign(x);  wb = sign(w)
    y1  = conv3x3(xb, wb1, pad=1)
    a1  = hardtanh(bn1(y1))          # only sign(a1) feeds conv2
    y2  = conv3x3(sign(a1), wb2, pad=1)
    out = hardtanh(bn2(y2) + x)

Strategy:
  - Data parallel: batch N=64 -> 8 images per core; weights/BN replicated.
  - Conv as 9 shifted matmuls over a zero-padded 58x58 image held in SBUF,
    contraction over input channels: 256 channels = 2 planes of 128
    partitions contracted in ONE matmul via fp8 DoubleRow perf mode.
  - Binarized operands stored as fp8e4 (+-1, 0 exact); PSUM accumulates
    fp32; sums of +-1 with <=2304 terms are exact integers in fp32.
  - BN folded into the activation op: sign(bn1(y)) = Sign(y*s1 + t1) with
    s1 = g1/sqrt(v1+eps), t1 = b1 - m1*s1 (host-folded, passed as inputs).
  - Final stage: Identity(y2*s2+t2) on ScalarE, then add-residual and
    clip (tensor_scalar min+max) on VectorE.

Schedule (the kernel is PE-bound at the fp8 DoubleRow ceiling of 1 px/cyc;
all that matters beyond the 381us matmul floor is head/tail latency):
  - Only sync (SP) and scalar (Activation) queues are hardware DGEs; the
    gpsimd queue is a software DGE with ~5us transfer latency and a slow
    final drain, so it only carries w2 + BN params (needed late, done
    early).
  - Image-0 input is split into 4 row-pieces per 128-ch plane on the sync
    queue so the first conv chunk's rows arrive ASAP; w1 loads
    concurrently on the scalar queue.
  - A short PE warm-up (junk matmuls) covers the input-DMA latency so the
    HAM is ramped when the first real matmul issues.
  - Pad cells of the double-buffered padded-image tiles are zeroed ONCE at
    the head on the vector engine (sign writes never touch them), not per
    image.
  - Outputs stored per 8-row chunk on the sync queue; the last chunk is
    split 6+2 rows so the final store's epilogue chain
    (identity -> add -> clip -> DMA) is short.
"""

import sys

try:
    import concourse  # noqa: F401
except ImportError:  # pragma: no cover
    sys.path.insert(0, "/opt/trn_rl_repo")

import numpy as np
import ml_dtypes

import concourse.bacc as bacc
import concourse.tile as tile
import concourse.mybir as mybir
from concourse.bass_utils import run_bass_kernel_spmd

dt = mybir.dt
AF = mybir.ActivationFunctionType
ALU = mybir.AluOpType
PM = mybir.MatmulPerfMode

N_CORES = 8
NPER = 8          # images per core
C = 256
H = W = 56
HW = H * W        # 3136
WP = 58           # padded row width (1 + 56 + 1)
ROWW = 64         # allocated width per (row, k-plane) block (16B aligned)
RPITCH = 2 * ROWW  # 128 = row pitch (both k-planes interleaved per row)
PROWS = 58        # padded rows
PLSZ = PROWS * RPITCH  # 7424 = padded image tile length
RPC = 8           # output rows per matmul chunk
CHU = RPC * W     # 448 = useful matmul free dim (garbage cols skipped)
NCH = H // RPC    # 7 chunks per image
BN_EPS = 1e-5
WARMUP = 13       # junk matmuls to ramp the PE while the first input loads
# conv2 output-row chunking: 6x8 rows + 6 + 2 (short tail chunk)
CH2 = ((0, 8), (8, 8), (16, 8), (24, 8), (32, 8), (40, 8), (48, 6), (54, 2))
CH1 = tuple((RPC * s, RPC) for s in range(NCH))
# center tap first: it always covers the full chunk, so it carries the
# PSUM-reset start flag; the edge taps may be truncated (pad skipping)
TAP_ORDER = (4, 0, 1, 2, 3, 5, 6, 7, 8)

_CACHE = {}


def _zero_pads(nc, t):
    """Zero the padding cells of a [128, PLSZ] row-interleaved image tile.

    Layout: element (row r, k-plane k, col c) at r*RPITCH + k*ROWW + c;
    c=1..56 hold image cols 0..55, c=0 and c=57..63 are zero pads, rows
    0 and 57 are zero pad rows."""
    v = t[:]
    nc.vector.memset(v[:, 0:RPITCH], 0.0)                      # top pad row
    nc.vector.memset(v[:, 57 * RPITCH:PLSZ], 0.0)              # bottom pad row
    # per-block right pads c=57..63 plus the following block's c=0
    cols = v[:, 57:57 + 57 * RPITCH].rearrange("p (r k c) -> p r k c", k=2, c=ROWW)
    nc.vector.memset(cols[:, :, :, 0:8], 0.0)


def _rview(t):
    # [128, PROWS, 2, ROWW]
    return t[:].rearrange("p (r k c) -> p r k c", k=2, c=ROWW)


def _conv_chunk(nc, ps_t, wt, src_v, ro, nr, co):
    """9-tap conv accumulation for output rows [ro, ro+nr).

    Pad rows/cols contribute zero, so the edge taps skip them instead of
    streaming them through the PE: the dv=0/2 taps drop the top/bottom pad
    row at the image border, the dh=0/2 taps drop the left/right pad
    column everywhere (~2% fewer PE cycles). kk=4 always covers the whole
    chunk and goes first with the PSUM-reset start flag."""
    psv = ps_t[:, 0:nr * W].rearrange("p (r c) -> p r c", c=W)
    for i, kk in enumerate(TAP_ORDER):
        dv, dh = kk // 3, kk % 3
        r_lo, r_hi = ro + dv, ro + dv + nr
        j_lo, j_hi = 0, nr
        if r_lo == 0:
            r_lo, j_lo = 1, 1
        if r_hi == PROWS:
            r_hi, j_hi = PROWS - 1, nr - 1
        c_lo, cw, oc = dh, W, 0
        if dh == 0:
            c_lo, cw, oc = 1, W - 1, 1
        elif dh == 2:
            cw = W - 1
        rhs = src_v[:, r_lo:r_hi, :, c_lo:c_lo + cw].rearrange("p r k c -> p k r c")
        nc.tensor.matmul(
            psv[:, j_lo:j_hi, oc:oc + cw],
            wt[:, :, kk, co * 128:(co + 1) * 128],
            rhs,
            start=(i == 0),
            stop=(i == 8),
            perf_mode=PM.DoubleRow,
        )


def _build():
    nc = bacc.Bacc("TRN2", target_bir_lowering=False, debug=False)

    x_d = nc.dram_tensor("x", [NPER, C, H, W], dt.float32, kind="ExternalInput").ap()
    w1_d = nc.dram_tensor("w1b", [2, 128, 9, C], dt.float8e4, kind="ExternalInput").ap()
    w2_d = nc.dram_tensor("w2b", [2, 128, 9, C], dt.float8e4, kind="ExternalInput").ap()
    # folded BN params packed [v, q, p] with v: 0=s1, 1=t1, 2=s2, 3=t2
    bn_d = nc.dram_tensor("bn", [4, 2, 128], dt.float32, kind="ExternalInput").ap()
    out_d = nc.dram_tensor("out", [NPER, C, H, W], dt.float32, kind="ExternalOutput").ap()

    with tile.TileContext(nc) as tc:
        with (
            tc.tile_pool(name="wp", bufs=1) as wp,
            tc.tile_pool(name="xin", bufs=4) as xinp,
            tc.tile_pool(name="ost", bufs=3) as ostp,
            tc.tile_pool(name="tmp", bufs=4) as tmpp,
            tc.tile_pool(name="ps", bufs=7, space="PSUM") as psp,
            nc.sbuf_tensor([128, PLSZ], dt.float8e4) as xb0,
            nc.sbuf_tensor([128, PLSZ], dt.float8e4) as xb1,
            nc.sbuf_tensor([128, PLSZ], dt.float8e4) as ab0,
            nc.sbuf_tensor([128, PLSZ], dt.float8e4) as ab1,
            nc.sbuf_tensor([128, 2 * CHU], dt.float8e4) as warm_in,
            nc.psum_tensor([128, CHU], dt.float32) as warm_ps,
        ):
            # ---- PE warm-up: junk matmuls on scratch data so the PE is
            # ramped (HAM) when the first real matmul issues; they overlap
            # the first input pieces' DMA+binarize latency. The scratch is
            # read uninitialized on purpose — results are discarded and a
            # memset would delay the first warm-up matmul.
            wv = warm_in[:].rearrange("p (k c) -> p k c", k=2)
            for _ in range(WARMUP):
                nc.tensor.matmul(
                    warm_ps[:], wv[:, :, 0:128], wv[:],
                    start=True, stop=True, perf_mode=PM.DoubleRow,
                )

            # Pad cells are read-only for matmuls and never overwritten by
            # the sign writes (cols 1..56 only): zero them once per buffer.
            for t in (xb0, ab0, xb1, ab1):
                _zero_pads(nc, t)

            # w1 heads the scalar queue (the first real matmul needs it);
            # image-0's last two q1 pieces share that queue. w2 + BN ride
            # the gpsimd software-DGE queue (high latency, needed late).
            w_sb = []
            for wd, tag in ((w1_d, "w1"), (w2_d, "w2")):
                t = wp.tile([128, 2, 9, C], dt.float8e4, tag=tag)
                w_sb.append(t)
            bn_sb = wp.tile([128, 8], dt.float32, tag="bn")
            nc.scalar.dma_start(w_sb[0][:], w1_d.rearrange("q p k c -> p q k c"))
            nc.gpsimd.dma_start(bn_sb[:], bn_d.rearrange("v q p -> p (v q)"))
            nc.gpsimd.dma_start(w_sb[1][:], w2_d.rearrange("q p k c -> p q k c"))

            for n in range(NPER):
                xb = (xb0, xb1)[n % 2]
                ab = (ab0, ab1)[n % 2]
                xbv = _rview(xb)
                abv = _rview(ab)

                # ---- load + binarize input ----
                xin = []
                for q in range(2):
                    xi = xinp.tile([128, HW], dt.float32, tag="xin")
                    xin.append(xi)

                def _load(q, r0, nr, dma_eng):
                    dma_eng.dma_start(
                        xin[q][:, r0 * W:(r0 + nr) * W],
                        x_d[n, q * 128:(q + 1) * 128, r0:r0 + nr].rearrange(
                            "p h w -> p (h w)"),
                    )

                def _sign_scalar(q, r0, nr):
                    nc.scalar.activation(
                        xbv[:, 1 + r0:1 + r0 + nr, q, 1:57],
                        xin[q][:, r0 * W:(r0 + nr) * W].rearrange(
                            "p (h w) -> p h w", w=W),
                        AF.Sign,
                    )

                def _sign_gpsimd(q, r0, nr):
                    # sign() emulated as (x > 0) * 2 - 1 so the binarize can
                    # run off the scalar engine during the kernel head
                    dst = xbv[:, 1 + r0:1 + r0 + nr, q, 1:57]
                    src = xin[q][:, r0 * W:(r0 + nr) * W].rearrange(
                        "p (h w) -> p h w", w=W)
                    nc.gpsimd.tensor_scalar(dst, src, 0.0, None, ALU.is_gt)
                    nc.gpsimd.tensor_scalar(dst, dst, 2.0, -1.0, ALU.mult, ALU.add)

                if n == 0:
                    # Image 0 is the kernel head: fine-grained row pieces,
                    # interleaved across the sync + scalar hw-DGE queues so
                    # the first conv chunks can start ASAP; the q1 planes
                    # binarize on gpsimd to keep the scalar engine free for
                    # the q0 signs right after its pushes + table load.
                    pieces = ((0, 10), (10, 12), (22, 12), (34, 10), (44, 12))
                    for pi, (r0, nr) in enumerate(pieces):
                        if pi == 0:
                            _load(0, r0, nr, nc.sync)
                            _load(1, r0, nr, nc.sync)
                        else:
                            _load(1, r0, nr, nc.sync if pi <= 2 else nc.scalar)
                            _load(0, r0, nr, nc.sync)
                    for r0, nr in pieces:
                        _sign_scalar(0, r0, nr)
                        _sign_gpsimd(1, r0, nr)
                else:
                    for r0, nr in ((0, 28), (28, 28)):
                        for q in range(2):
                            _load(q, r0, nr, nc.sync)
                            _sign_scalar(q, r0, nr)

                # ---- conv1 -> sign(bn1(.)) into padded intermediate ----
                for co in range(2):
                    for ro, nr in CH1:
                        ps = psp.tile([128, CHU], dt.float32, tag="ps")
                        _conv_chunk(nc, ps, w_sb[0], xbv, ro, nr, co)
                        psv = ps[:, 0:nr * W].rearrange("p (r c) -> p r c", c=W)
                        nc.scalar.activation(
                            abv[:, 1 + ro:1 + ro + nr, co, 1:57], psv, AF.Sign,
                            bias=bn_sb[:, 2 + co:3 + co], scale=bn_sb[:, 0 + co:1 + co],
                        )

                # ---- conv2 -> bn2 + residual + clip -> store per chunk ----
                for co in range(2):
                    ost = ostp.tile([128, HW], dt.float32, tag="ost")
                    ostv = ost[:].rearrange("p (h w) -> p h w", w=W)
                    xinv = xin[co][:].rearrange("p (h w) -> p h w", w=W)
                    for ro, nr in CH2:
                        fd = nr * W
                        pst = psp.tile([128, CHU], dt.float32, tag="ps")
                        _conv_chunk(nc, pst, w_sb[1], abv, ro, nr, co)
                        psv = pst[:, 0:fd].rearrange("p (r c) -> p r c", c=W)
                        tm = tmpp.tile([128, CHU], dt.float32, tag="tmp")
                        tmv = tm[:, 0:fd].rearrange("p (r c) -> p r c", c=W)
                        nc.scalar.activation(
                            tmv, psv, AF.Identity,
                            bias=bn_sb[:, 6 + co:7 + co], scale=bn_sb[:, 4 + co:5 + co],
                        )
                        ov = ostv[:, ro:ro + nr, :]
                        nc.vector.tensor_tensor(
                            ov, tmv, xinv[:, ro:ro + nr, :], ALU.add
                        )
                        nc.vector.tensor_scalar(ov, ov, 1.0, -1.0, ALU.min, ALU.max)
                        nc.sync.dma_start(
                            out_d[n, co * 128:(co + 1) * 128, ro:ro + nr].rearrange(
                                "p h w -> p (h w)"),
                            ost[:, ro * W:(ro + nr) * W],
                        )

    nc.compile()
    return nc


def _get_nc():
    if "nc" not in _CACHE:
        _CACHE["nc"] = _build()
    return _CACHE["nc"]


def _prep_weights(w):
    # [co, cin, kh, kw] -> [cin_chunk 2, cin 128, tap 9, co 256], binarized fp8e4
    a = np.sign(w.astype(np.float32))
    a = a.transpose(1, 2, 3, 0).reshape(2, 128, 9, C)
    return np.ascontiguousarray(a.astype(ml_dtypes.float8_e4m3))


def _fold_bn(g, b, m, v):
    s = (g.astype(np.float32) / np.sqrt(v.astype(np.float32) + BN_EPS)).astype(np.float32)
    t = (b.astype(np.float32) - m.astype(np.float32) * s).astype(np.float32)
    return s.reshape(2, 128), t.reshape(2, 128)


def _prep_in_maps(x, w1, g1, b1, m1, v1, w2, g2, b2, m2, v2):
    w1b = _prep_weights(w1)
    w2b = _prep_weights(w2)
    s1, t1 = _fold_bn(g1, b1, m1, v1)
    s2, t2 = _fold_bn(g2, b2, m2, v2)
    bn = np.ascontiguousarray(np.stack([s1, t1, s2, t2]))  # [4, 2, 128]
    x = np.ascontiguousarray(x.astype(np.float32, copy=False))
    return [{
        "x": x[c * NPER:(c + 1) * NPER],
        "w1b": w1b, "w2b": w2b, "bn": bn,
    } for c in range(N_CORES)]


def kernel(x, w1, g1, b1, m1, v1, w2, g2, b2, m2, v2):
    nc = _get_nc()
    in_maps = _prep_in_maps(x, w1, g1, b1, m1, v1, w2, g2, b2, m2, v2)
    res = run_bass_kernel_spmd(nc, in_maps, list(range(N_CORES)))
    out = np.concatenate([res.results[c]["out"] for c in range(N_CORES)], axis=0)
    return out


# revision 19
# speedup vs baseline: 1.0986x; 1.0986x over previous
"""Binarized BasicBlock (BNN) forward on 8 Trainium2 NeuronCores.

Reference computation (per reference.py):
    xb  = sign(x);  wb = sign(w)
    y1  = conv3x3(xb, wb1, pad=1)
    a1  = hardtanh(bn1(y1))          # only sign(a1) feeds conv2
    y2  = conv3x3(sign(a1), wb2, pad=1)
    out = hardtanh(bn2(y2) + x)

Strategy:
  - Data parallel: batch N=64 -> 8 images per core; weights/BN replicated.
  - Conv as 9 shifted matmuls over a zero-padded 58x58 image held in SBUF,
    contraction over input channels: 256 channels = 2 planes of 128
    partitions contracted in ONE matmul via fp8 DoubleRow perf mode.
  - Binarized operands stored as fp8e4 (+-1, 0 exact); PSUM accumulates
    fp32; sums of +-1 with <=2304 terms are exact integers in fp32.
  - BN folded into the activation op: sign(bn1(y)) = Sign(y*s1 + t1) with
    s1 = g1/sqrt(v1+eps), t1 = b1 - m1*s1 (host-folded, passed as inputs).
  - Final stage: Identity(y2*s2+t2) on ScalarE, then add-residual and
    clip (tensor_scalar min+max) on VectorE.

Schedule (the kernel is PE-bound at the fp8 DoubleRow ceiling of 1 px/cyc;
all that matters beyond the 381us matmul floor is head/tail latency):
  - Only sync (SP) and scalar (Activation) queues are hardware DGEs; the
    gpsimd queue is a software DGE with ~5us transfer latency and a slow
    final drain, so it only carries w2 + BN params (needed late, done
    early).
  - Image-0 input is split into 4 row-pieces per 128-ch plane on the sync
    queue so the first conv chunk's rows arrive ASAP; w1 loads
    concurrently on the scalar queue.
  - A short PE warm-up (junk matmuls) covers the input-DMA latency so the
    HAM is ramped when the first real matmul issues.
  - Pad cells of the double-buffered padded-image tiles are zeroed ONCE at
    the head on the vector engine (sign writes never touch them), not per
    image.
  - Outputs stored per 8-row chunk on the sync queue; the last chunk is
    split 6+2 rows so the final store's epilogue chain
    (identity -> add -> clip -> DMA) is short.
"""

import sys

try:
    import concourse  # noqa: F401
except ImportError:  # pragma: no cover
    sys.path.insert(0, "/opt/trn_rl_repo")

import numpy as np
import ml_dtypes

import concourse.bacc as bacc
import concourse.tile as tile
import concourse.mybir as mybir
from concourse.bass_utils import run_bass_kernel_spmd

dt = mybir.dt
AF = mybir.ActivationFunctionType
ALU = mybir.AluOpType
PM = mybir.MatmulPerfMode

N_CORES = 8
NPER = 8          # images per core
C = 256
H = W = 56
HW = H * W        # 3136
WP = 58           # padded row width (1 + 56 + 1)
ROWW = 64         # allocated width per (row, k-plane) block (16B aligned)
RPITCH = 2 * ROWW  # 128 = row pitch (both k-planes interleaved per row)
PROWS = 58        # padded rows
PLSZ = PROWS * RPITCH  # 7424 = padded image tile length
RPC = 8           # output rows per matmul chunk
CHU = RPC * W     # 448 = useful matmul free dim (garbage cols skipped)
NCH = H // RPC    # 7 chunks per image
BN_EPS = 1e-5
WARMUP = 13       # junk matmuls to ramp the PE while the first input loads
# conv2 output-row chunking: 6x8 rows + 6 + 2 (short tail chunk)
CH2 = ((0, 8), (8, 8), (16, 8), (24, 8), (32, 8), (40, 8), (48, 6), (54, 2))
CH1 = tuple((RPC * s, RPC) for s in range(NCH))
# center tap first: it always covers the full chunk, so it carries the
# PSUM-reset start flag; the edge taps may be truncated (pad skipping)
TAP_ORDER = (4, 0, 1, 2, 3, 5, 6, 7, 8)

_CACHE = {}


def _zero_pads(nc, t):
    """Zero the padding cells of a [128, PLSZ] row-interleaved image tile.

    Layout: element (row r, k-plane k, col c) at r*RPITCH + k*ROWW + c;
    c=1..56 hold image cols 0..55, c=0 and c=57..63 are zero pads, rows
    0 and 57 are zero pad rows."""
    v = t[:]
    nc.vector.memset(v[:, 0:RPITCH], 0.0)                      # top pad row
    nc.vector.memset(v[:, 57 * RPITCH:PLSZ], 0.0)              # bottom pad row
    # per-block right pads c=57..63 plus the following block's c=0
    cols = v[:, 57:57 + 57 * RPITCH].rearrange("p (r k c) -> p r k c", k=2, c=ROWW)
    nc.vector.memset(cols[:, :, :, 0:8], 0.0)


def _rview(t):
    # [128, PROWS, 2, ROWW]
    return t[:].rearrange("p (r k c) -> p r k c", k=2, c=ROWW)


def _conv_chunk(nc, ps_t, wt, src_v, ro, nr, co):
    """9-tap conv accumulation for output rows [ro, ro+nr).

    Pad rows/cols contribute zero, so the edge taps skip them instead of
    streaming them through the PE: the dv=0/2 taps drop the top/bottom pad
    row at the image border, the dh=0/2 taps drop the left/right pad
    column everywhere (~2% fewer PE cycles). kk=4 always covers the whole
    chunk and goes first with the PSUM-reset start flag."""
    psv = ps_t[:, 0:nr * W].rearrange("p (r c) -> p r c", c=W)
    for i, kk in enumerate(TAP_ORDER):
        dv, dh = kk // 3, kk % 3
        r_lo, r_hi = ro + dv, ro + dv + nr
        j_lo, j_hi = 0, nr
        if r_lo == 0:
            r_lo, j_lo = 1, 1
        if r_hi == PROWS:
            r_hi, j_hi = PROWS - 1, nr - 1
        c_lo, cw, oc = dh, W, 0
        if dh == 0:
            c_lo, cw, oc = 1, W - 1, 1
        elif dh == 2:
            cw = W - 1
        rhs = src_v[:, r_lo:r_hi, :, c_lo:c_lo + cw].rearrange("p r k c -> p k r c")
        nc.tensor.matmul(
            psv[:, j_lo:j_hi, oc:oc + cw],
            wt[:, :, kk, co * 128:(co + 1) * 128],
            rhs,
            start=(i == 0),
            stop=(i == 8),
            perf_mode=PM.DoubleRow,
        )


def _build():
    nc = bacc.Bacc("TRN2", target_bir_lowering=False, debug=False)

    x_d = nc.dram_tensor("x", [NPER, C, H, W], dt.float32, kind="ExternalInput").ap()
    w1_d = nc.dram_tensor("w1b", [2, 128, 9, C], dt.float8e4, kind="ExternalInput").ap()
    w2_d = nc.dram_tensor("w2b", [2, 128, 9, C], dt.float8e4, kind="ExternalInput").ap()
    # folded BN params packed [v, q, p] with v: 0=s1, 1=t1, 2=s2, 3=t2
    bn_d = nc.dram_tensor("bn", [4, 2, 128], dt.float32, kind="ExternalInput").ap()
    out_d = nc.dram_tensor("out", [NPER, C, H, W], dt.float32, kind="ExternalOutput").ap()

    with tile.TileContext(nc) as tc:
        with (
            tc.tile_pool(name="wp", bufs=1) as wp,
            tc.tile_pool(name="xin", bufs=4) as xinp,
            tc.tile_pool(name="ost", bufs=3) as ostp,
            tc.tile_pool(name="tmp", bufs=4) as tmpp,
            tc.tile_pool(name="ps", bufs=7, space="PSUM") as psp,
            nc.sbuf_tensor([128, PLSZ], dt.float8e4) as xb0,
            nc.sbuf_tensor([128, PLSZ], dt.float8e4) as xb1,
            nc.sbuf_tensor([128, PLSZ], dt.float8e4) as ab0,
            nc.sbuf_tensor([128, PLSZ], dt.float8e4) as ab1,
            nc.sbuf_tensor([128, 2 * CHU], dt.float8e4) as warm_in,
            nc.psum_tensor([128, CHU], dt.float32) as warm_ps,
        ):
            # ---- PE warm-up: junk matmuls on scratch data so the PE is
            # ramped (HAM) when the first real matmul issues; they overlap
            # the first input pieces' DMA+binarize latency. The scratch is
            # read uninitialized on purpose — results are discarded and a
            # memset would delay the first warm-up matmul.
            wv = warm_in[:].rearrange("p (k c) -> p k c", k=2)
            for _ in range(WARMUP):
                nc.tensor.matmul(
                    warm_ps[:], wv[:, :, 0:128], wv[:],
                    start=True, stop=True, perf_mode=PM.DoubleRow,
                )

            # Pad cells are read-only for matmuls and never overwritten by
            # the sign writes (cols 1..56 only): zero them once per buffer.
            # Only xb0's pads gate the first matmul; the other three tiles'
            # pads are emitted after image 0's binarize (vector engine order)
            # since they are not read until image 0's conv2 / image 1.
            _zero_pads(nc, xb0)

            # w1 heads the scalar queue (the first real matmul needs it);
            # image-0's last two q1 pieces share that queue. w2 + BN ride
            # the gpsimd software-DGE queue (high latency, needed late).
            w_sb = []
            for wd, tag in ((w1_d, "w1"), (w2_d, "w2")):
                t = wp.tile([128, 2, 9, C], dt.float8e4, tag=tag)
                w_sb.append(t)
            bn_sb = wp.tile([128, 8], dt.float32, tag="bn")
            nc.scalar.dma_start(w_sb[0][:], w1_d.rearrange("q p k c -> p q k c"))
            nc.gpsimd.dma_start(bn_sb[:], bn_d.rearrange("v q p -> p (v q)"))
            nc.gpsimd.dma_start(w_sb[1][:], w2_d.rearrange("q p k c -> p q k c"))

            for n in range(NPER):
                xb = (xb0, xb1)[n % 2]
                ab = (ab0, ab1)[n % 2]
                xbv = _rview(xb)
                abv = _rview(ab)

                # ---- load + binarize input ----
                xin = []
                for q in range(2):
                    xi = xinp.tile([128, HW], dt.float32, tag="xin")
                    xin.append(xi)

                def _load(q, r0, nr, dma_eng):
                    dma_eng.dma_start(
                        xin[q][:, r0 * W:(r0 + nr) * W],
                        x_d[n, q * 128:(q + 1) * 128, r0:r0 + nr].rearrange(
                            "p h w -> p (h w)"),
                    )

                def _sign_scalar(q, r0, nr):
                    nc.scalar.activation(
                        xbv[:, 1 + r0:1 + r0 + nr, q, 1:57],
                        xin[q][:, r0 * W:(r0 + nr) * W].rearrange(
                            "p (h w) -> p h w", w=W),
                        AF.Sign,
                    )

                def _sign_vector(q, r0, nr):
                    # sign() emulated as (x > 0) * 2 - 1 so the binarize can
                    # run off the scalar engine during the kernel head
                    dst = xbv[:, 1 + r0:1 + r0 + nr, q, 1:57]
                    src = xin[q][:, r0 * W:(r0 + nr) * W].rearrange(
                        "p (h w) -> p h w", w=W)
                    nc.vector.tensor_scalar(dst, src, 0.0, None, ALU.is_gt)
                    nc.vector.tensor_scalar(dst, dst, 2.0, -1.0, ALU.mult, ALU.add)

                if n == 0:
                    # Image 0 is the kernel head: fine-grained row pieces,
                    # interleaved across the sync + scalar hw-DGE queues so
                    # the first conv chunks can start ASAP; the q1 planes
                    # binarize on gpsimd to keep the scalar engine free for
                    # the q0 signs right after its pushes + table load.
                    pieces = ((0, 10), (10, 12), (22, 12), (34, 10), (44, 12))
                    for pi, (r0, nr) in enumerate(pieces):
                        if pi == 0:
                            _load(0, r0, nr, nc.sync)
                            _load(1, r0, nr, nc.sync)
                        else:
                            _load(1, r0, nr, nc.sync if pi <= 2 else nc.scalar)
                            _load(0, r0, nr, nc.sync)
                    for r0, nr in pieces:
                        _sign_scalar(0, r0, nr)
                        _sign_vector(1, r0, nr)
                    for t in (ab0, xb1, ab1):
                        _zero_pads(nc, t)
                else:
                    for r0, nr in ((0, 28), (28, 28)):
                        for q in range(2):
                            _load(q, r0, nr, nc.sync)
                            _sign_scalar(q, r0, nr)

                # ---- conv1 -> sign(bn1(.)) into padded intermediate ----
                for co in range(2):
                    for ro, nr in CH1:
                        ps = psp.tile([128, CHU], dt.float32, tag="ps")
                        _conv_chunk(nc, ps, w_sb[0], xbv, ro, nr, co)
                        psv = ps[:, 0:nr * W].rearrange("p (r c) -> p r c", c=W)
                        nc.scalar.activation(
                            abv[:, 1 + ro:1 + ro + nr, co, 1:57], psv, AF.Sign,
                            bias=bn_sb[:, 2 + co:3 + co], scale=bn_sb[:, 0 + co:1 + co],
                        )

                # ---- conv2 -> bn2 + residual + clip -> store per chunk ----
                for co in range(2):
                    ost = ostp.tile([128, HW], dt.float32, tag="ost")
                    ostv = ost[:].rearrange("p (h w) -> p h w", w=W)
                    xinv = xin[co][:].rearrange("p (h w) -> p h w", w=W)
                    for ro, nr in CH2:
                        fd = nr * W
                        pst = psp.tile([128, CHU], dt.float32, tag="ps")
                        _conv_chunk(nc, pst, w_sb[1], abv, ro, nr, co)
                        psv = pst[:, 0:fd].rearrange("p (r c) -> p r c", c=W)
                        tm = tmpp.tile([128, CHU], dt.float32, tag="tmp")
                        tmv = tm[:, 0:fd].rearrange("p (r c) -> p r c", c=W)
                        nc.scalar.activation(
                            tmv, psv, AF.Identity,
                            bias=bn_sb[:, 6 + co:7 + co], scale=bn_sb[:, 4 + co:5 + co],
                        )
                        ov = ostv[:, ro:ro + nr, :]
                        nc.vector.tensor_tensor(
                            ov, tmv, xinv[:, ro:ro + nr, :], ALU.add
                        )
                        nc.vector.tensor_scalar(ov, ov, 1.0, -1.0, ALU.min, ALU.max)
                        nc.sync.dma_start(
                            out_d[n, co * 128:(co + 1) * 128, ro:ro + nr].rearrange(
                                "p h w -> p (h w)"),
                            ost[:, ro * W:(ro + nr) * W],
                        )

    nc.compile()
    return nc


def _get_nc():
    if "nc" not in _CACHE:
        _CACHE["nc"] = _build()
    return _CACHE["nc"]


def _prep_weights(w):
    # [co, cin, kh, kw] -> [cin_chunk 2, cin 128, tap 9, co 256], binarized fp8e4
    a = np.sign(w.astype(np.float32))
    a = a.transpose(1, 2, 3, 0).reshape(2, 128, 9, C)
    return np.ascontiguousarray(a.astype(ml_dtypes.float8_e4m3))


def _fold_bn(g, b, m, v):
    s = (g.astype(np.float32) / np.sqrt(v.astype(np.float32) + BN_EPS)).astype(np.float32)
    t = (b.astype(np.float32) - m.astype(np.float32) * s).astype(np.float32)
    return s.reshape(2, 128), t.reshape(2, 128)


def _prep_in_maps(x, w1, g1, b1, m1, v1, w2, g2, b2, m2, v2):
    w1b = _prep_weights(w1)
    w2b = _prep_weights(w2)
    s1, t1 = _fold_bn(g1, b1, m1, v1)
    s2, t2 = _fold_bn(g2, b2, m2, v2)
    bn = np.ascontiguousarray(np.stack([s1, t1, s2, t2]))  # [4, 2, 128]
    x = np.ascontiguousarray(x.astype(np.float32, copy=False))
    return [{
        "x": x[c * NPER:(c + 1) * NPER],
        "w1b": w1b, "w2b": w2b, "bn": bn,
    } for c in range(N_CORES)]


def kernel(x, w1, g1, b1, m1, v1, w2, g2, b2, m2, v2):
    nc = _get_nc()
    in_maps = _prep_in_maps(x, w1, g1, b1, m1, v1, w2, g2, b2, m2, v2)
    res = run_bass_kernel_spmd(nc, in_maps, list(range(N_CORES)))
    out = np.concatenate([res.results[c]["out"] for c in range(N_CORES)], axis=0)
    return out


# revision 20
# speedup vs baseline: 1.1087x; 1.0091x over previous
"""Binarized BasicBlock (BNN) forward on 8 Trainium2 NeuronCores.

Reference computation (per reference.py):
    xb  = sign(x);  wb = sign(w)
    y1  = conv3x3(xb, wb1, pad=1)
    a1  = hardtanh(bn1(y1))          # only sign(a1) feeds conv2
    y2  = conv3x3(sign(a1), wb2, pad=1)
    out = hardtanh(bn2(y2) + x)

Strategy:
  - Data parallel: batch N=64 -> 8 images per core; weights/BN replicated.
  - Conv as 9 shifted matmuls over a zero-padded 58x58 image held in SBUF,
    contraction over input channels: 256 channels = 2 planes of 128
    partitions contracted in ONE matmul via fp8 DoubleRow perf mode.
  - Binarized operands stored as fp8e4 (+-1, 0 exact); PSUM accumulates
    fp32; sums of +-1 with <=2304 terms are exact integers in fp32.
  - BN folded into the activation op: sign(bn1(y)) = Sign(y*s1 + t1) with
    s1 = g1/sqrt(v1+eps), t1 = b1 - m1*s1 (host-folded, passed as inputs).
  - Final stage: Identity(y2*s2+t2) on ScalarE, then add-residual and
    clip (tensor_scalar min+max) on VectorE.

Schedule (the kernel is PE-bound at the fp8 DoubleRow ceiling of 1 px/cyc;
all that matters beyond the 381us matmul floor is head/tail latency):
  - Only sync (SP) and scalar (Activation) queues are hardware DGEs; the
    gpsimd queue is a software DGE with ~5us transfer latency and a slow
    final drain, so it only carries w2 + BN params (needed late, done
    early).
  - Image-0 input is split into 4 row-pieces per 128-ch plane on the sync
    queue so the first conv chunk's rows arrive ASAP; w1 loads
    concurrently on the scalar queue.
  - A short PE warm-up (junk matmuls) covers the input-DMA latency so the
    HAM is ramped when the first real matmul issues.
  - Pad cells of the double-buffered padded-image tiles are zeroed ONCE at
    the head on the vector engine (sign writes never touch them), not per
    image.
  - Outputs stored per 8-row chunk on the sync queue; the last chunk is
    split 6+2 rows so the final store's epilogue chain
    (identity -> add -> clip -> DMA) is short.
"""

import sys

try:
    import concourse  # noqa: F401
except ImportError:  # pragma: no cover
    sys.path.insert(0, "/opt/trn_rl_repo")

import numpy as np
import ml_dtypes

import concourse.bacc as bacc
import concourse.tile as tile
import concourse.mybir as mybir
from concourse.bass_utils import run_bass_kernel_spmd

dt = mybir.dt
AF = mybir.ActivationFunctionType
ALU = mybir.AluOpType
PM = mybir.MatmulPerfMode

N_CORES = 8
NPER = 8          # images per core
C = 256
H = W = 56
HW = H * W        # 3136
WP = 58           # padded row width (1 + 56 + 1)
ROWW = 64         # allocated width per (row, k-plane) block (16B aligned)
RPITCH = 2 * ROWW  # 128 = row pitch (both k-planes interleaved per row)
PROWS = 58        # padded rows
PLSZ = PROWS * RPITCH  # 7424 = padded image tile length
RPC = 8           # output rows per matmul chunk
CHU = RPC * W     # 448 = useful matmul free dim (garbage cols skipped)
NCH = H // RPC    # 7 chunks per image
BN_EPS = 1e-5
WARMUP = 13       # junk matmuls to ramp the PE while the first input loads
# conv2 output-row chunking: 6x8 rows + 6 + 2 (short tail chunk)
CH2 = ((0, 8), (8, 8), (16, 8), (24, 8), (32, 8), (40, 8), (48, 6), (54, 2))
CH1 = tuple((RPC * s, RPC) for s in range(NCH))
# center tap first: it always covers the full chunk, so it carries the
# PSUM-reset start flag; the edge taps may be truncated (pad skipping)
TAP_ORDER = (4, 0, 1, 2, 3, 5, 6, 7, 8)

_CACHE = {}


def _zero_pads(nc, t):
    """Zero the padding cells of a [128, PLSZ] row-interleaved image tile.

    Layout: element (row r, k-plane k, col c) at r*RPITCH + k*ROWW + c;
    c=1..56 hold image cols 0..55, c=0 and c=57..63 are zero pads, rows
    0 and 57 are zero pad rows."""
    v = t[:]
    nc.vector.memset(v[:, 0:RPITCH], 0.0)                      # top pad row
    nc.vector.memset(v[:, 57 * RPITCH:PLSZ], 0.0)              # bottom pad row
    # per-block right pads c=57..63 plus the following block's c=0
    cols = v[:, 57:57 + 57 * RPITCH].rearrange("p (r k c) -> p r k c", k=2, c=ROWW)
    nc.vector.memset(cols[:, :, :, 0:8], 0.0)


def _rview(t):
    # [128, PROWS, 2, ROWW]
    return t[:].rearrange("p (r k c) -> p r k c", k=2, c=ROWW)


def _conv_chunk(nc, ps_t, wt, src_v, ro, nr, co):
    """9-tap conv accumulation for output rows [ro, ro+nr).

    Pad rows/cols contribute zero, so the edge taps skip them instead of
    streaming them through the PE: the dv=0/2 taps drop the top/bottom pad
    row at the image border, the dh=0/2 taps drop the left/right pad
    column everywhere (~2% fewer PE cycles). kk=4 always covers the whole
    chunk and goes first with the PSUM-reset start flag."""
    psv = ps_t[:, 0:nr * W].rearrange("p (r c) -> p r c", c=W)
    for i, kk in enumerate(TAP_ORDER):
        dv, dh = kk // 3, kk % 3
        r_lo, r_hi = ro + dv, ro + dv + nr
        j_lo, j_hi = 0, nr
        if r_lo == 0:
            r_lo, j_lo = 1, 1
        if r_hi == PROWS:
            r_hi, j_hi = PROWS - 1, nr - 1
        c_lo, cw, oc = dh, W, 0
        if dh == 0:
            c_lo, cw, oc = 1, W - 1, 1
        elif dh == 2:
            cw = W - 1
        rhs = src_v[:, r_lo:r_hi, :, c_lo:c_lo + cw].rearrange("p r k c -> p k r c")
        nc.tensor.matmul(
            psv[:, j_lo:j_hi, oc:oc + cw],
            wt[:, :, kk, co * 128:(co + 1) * 128],
            rhs,
            start=(i == 0),
            stop=(i == 8),
            perf_mode=PM.DoubleRow,
        )


def _build():
    nc = bacc.Bacc("TRN2", target_bir_lowering=False, debug=False)

    x_d = nc.dram_tensor("x", [NPER, C, H, W], dt.float32, kind="ExternalInput").ap()
    w1_d = nc.dram_tensor("w1b", [2, 128, 9, C], dt.float8e4, kind="ExternalInput").ap()
    w2_d = nc.dram_tensor("w2b", [2, 128, 9, C], dt.float8e4, kind="ExternalInput").ap()
    # folded BN params packed [v, q, p] with v: 0=s1, 1=t1, 2=s2, 3=t2
    bn_d = nc.dram_tensor("bn", [4, 2, 128], dt.float32, kind="ExternalInput").ap()
    out_d = nc.dram_tensor("out", [NPER, C, H, W], dt.float32, kind="ExternalOutput").ap()

    with tile.TileContext(nc) as tc:
        with (
            tc.tile_pool(name="wp", bufs=1) as wp,
            tc.tile_pool(name="xin", bufs=4) as xinp,
            tc.tile_pool(name="ost", bufs=3) as ostp,
            tc.tile_pool(name="tmp", bufs=4) as tmpp,
            tc.tile_pool(name="ps", bufs=7, space="PSUM") as psp,
            nc.sbuf_tensor([128, PLSZ], dt.float8e4) as xb0,
            nc.sbuf_tensor([128, PLSZ], dt.float8e4) as xb1,
            nc.sbuf_tensor([128, PLSZ], dt.float8e4) as ab0,
            nc.sbuf_tensor([128, PLSZ], dt.float8e4) as ab1,
            nc.sbuf_tensor([128, 2 * CHU], dt.float8e4) as warm_in,
            nc.psum_tensor([128, CHU], dt.float32) as warm_ps,
        ):
            # ---- PE warm-up: junk matmuls on scratch data so the PE is
            # ramped (HAM) when the first real matmul issues; they overlap
            # the first input pieces' DMA+binarize latency. The scratch is
            # read uninitialized on purpose — results are discarded and a
            # memset would delay the first warm-up matmul.
            wv = warm_in[:].rearrange("p (k c) -> p k c", k=2)
            for _ in range(WARMUP):
                nc.tensor.matmul(
                    warm_ps[:], wv[:, :, 0:128], wv[:],
                    start=True, stop=True, perf_mode=PM.DoubleRow,
                )

            # Pad cells are read-only for matmuls and never overwritten by
            # the sign writes (cols 1..56 only): zero them once per buffer.
            # Only xb0's pads gate the first matmul; the other three tiles'
            # pads are emitted after image 0's binarize (vector engine order)
            # since they are not read until image 0's conv2 / image 1.
            _zero_pads(nc, xb0)

            # w1 heads the scalar queue (the first real matmul needs it);
            # image-0's last two q1 pieces share that queue. w2 + BN ride
            # the gpsimd software-DGE queue (high latency, needed late).
            w_sb = []
            for wd, tag in ((w1_d, "w1"), (w2_d, "w2")):
                t = wp.tile([128, 2, 9, C], dt.float8e4, tag=tag)
                w_sb.append(t)
            bn_sb = wp.tile([128, 8], dt.float32, tag="bn")
            nc.scalar.dma_start(w_sb[0][:], w1_d.rearrange("q p k c -> p q k c"))
            nc.gpsimd.dma_start(bn_sb[:], bn_d.rearrange("v q p -> p (v q)"))
            nc.gpsimd.dma_start(w_sb[1][:], w2_d.rearrange("q p k c -> p q k c"))

            for n in range(NPER):
                xb = (xb0, xb1)[n % 2]
                ab = (ab0, ab1)[n % 2]
                xbv = _rview(xb)
                abv = _rview(ab)

                # ---- load + binarize input ----
                xin = []
                for q in range(2):
                    xi = xinp.tile([128, HW], dt.float32, tag="xin")
                    xin.append(xi)

                def _load(q, r0, nr, dma_eng):
                    dma_eng.dma_start(
                        xin[q][:, r0 * W:(r0 + nr) * W],
                        x_d[n, q * 128:(q + 1) * 128, r0:r0 + nr].rearrange(
                            "p h w -> p (h w)"),
                    )

                def _sign_scalar(q, r0, nr):
                    nc.scalar.activation(
                        xbv[:, 1 + r0:1 + r0 + nr, q, 1:57],
                        xin[q][:, r0 * W:(r0 + nr) * W].rearrange(
                            "p (h w) -> p h w", w=W),
                        AF.Sign,
                    )

                def _sign_vector(q, r0, nr):
                    # sign() emulated as (x > 0) * 2 - 1 so the binarize can
                    # run off the scalar engine during the kernel head
                    dst = xbv[:, 1 + r0:1 + r0 + nr, q, 1:57]
                    src = xin[q][:, r0 * W:(r0 + nr) * W].rearrange(
                        "p (h w) -> p h w", w=W)
                    nc.vector.tensor_scalar(dst, src, 0.0, None, ALU.is_gt)
                    nc.vector.tensor_scalar(dst, dst, 2.0, -1.0, ALU.mult, ALU.add)

                if n == 0:
                    # Image 0 is the kernel head, and it is HBM-bandwidth
                    # bound (~11us of wire time for image + w1): stream the
                    # input in row order, q0/q1 interleaved on the sync
                    # queue, so conv1's chunks unlock as rows arrive. The
                    # q1 planes binarize on the vector engine to halve the
                    # scalar engine's head-critical sign latency.
                    pieces = ((0, 10), (10, 12), (22, 12), (34, 10), (44, 12))
                    for r0, nr in pieces:
                        _load(0, r0, nr, nc.sync)
                        _load(1, r0, nr, nc.sync)
                    for r0, nr in pieces:
                        _sign_scalar(0, r0, nr)
                        _sign_vector(1, r0, nr)
                    for t in (ab0, xb1, ab1):
                        _zero_pads(nc, t)
                else:
                    for r0, nr in ((0, 28), (28, 28)):
                        for q in range(2):
                            _load(q, r0, nr, nc.sync)
                            _sign_scalar(q, r0, nr)

                # ---- conv1 -> sign(bn1(.)) into padded intermediate ----
                for co in range(2):
                    for ro, nr in CH1:
                        ps = psp.tile([128, CHU], dt.float32, tag="ps")
                        _conv_chunk(nc, ps, w_sb[0], xbv, ro, nr, co)
                        psv = ps[:, 0:nr * W].rearrange("p (r c) -> p r c", c=W)
                        nc.scalar.activation(
                            abv[:, 1 + ro:1 + ro + nr, co, 1:57], psv, AF.Sign,
                            bias=bn_sb[:, 2 + co:3 + co], scale=bn_sb[:, 0 + co:1 + co],
                        )

                # ---- conv2 -> bn2 + residual + clip -> store per chunk ----
                for co in range(2):
                    ost = ostp.tile([128, HW], dt.float32, tag="ost")
                    ostv = ost[:].rearrange("p (h w) -> p h w", w=W)
                    xinv = xin[co][:].rearrange("p (h w) -> p h w", w=W)
                    for ro, nr in CH2:
                        fd = nr * W
                        pst = psp.tile([128, CHU], dt.float32, tag="ps")
                        _conv_chunk(nc, pst, w_sb[1], abv, ro, nr, co)
                        psv = pst[:, 0:fd].rearrange("p (r c) -> p r c", c=W)
                        tm = tmpp.tile([128, CHU], dt.float32, tag="tmp")
                        tmv = tm[:, 0:fd].rearrange("p (r c) -> p r c", c=W)
                        nc.scalar.activation(
                            tmv, psv, AF.Identity,
                            bias=bn_sb[:, 6 + co:7 + co], scale=bn_sb[:, 4 + co:5 + co],
                        )
                        ov = ostv[:, ro:ro + nr, :]
                        nc.vector.tensor_tensor(
                            ov, tmv, xinv[:, ro:ro + nr, :], ALU.add
                        )
                        nc.vector.tensor_scalar(ov, ov, 1.0, -1.0, ALU.min, ALU.max)
                        nc.sync.dma_start(
                            out_d[n, co * 128:(co + 1) * 128, ro:ro + nr].rearrange(
                                "p h w -> p (h w)"),
                            ost[:, ro * W:(ro + nr) * W],
                        )

    nc.compile()
    return nc


def _get_nc():
    if "nc" not in _CACHE:
        _CACHE["nc"] = _build()
    return _CACHE["nc"]


def _prep_weights(w):
    # [co, cin, kh, kw] -> [cin_chunk 2, cin 128, tap 9, co 256], binarized fp8e4
    a = np.sign(w.astype(np.float32))
    a = a.transpose(1, 2, 3, 0).reshape(2, 128, 9, C)
    return np.ascontiguousarray(a.astype(ml_dtypes.float8_e4m3))


def _fold_bn(g, b, m, v):
    s = (g.astype(np.float32) / np.sqrt(v.astype(np.float32) + BN_EPS)).astype(np.float32)
    t = (b.astype(np.float32) - m.astype(np.float32) * s).astype(np.float32)
    return s.reshape(2, 128), t.reshape(2, 128)


def _prep_in_maps(x, w1, g1, b1, m1, v1, w2, g2, b2, m2, v2):
    w1b = _prep_weights(w1)
    w2b = _prep_weights(w2)
    s1, t1 = _fold_bn(g1, b1, m1, v1)
    s2, t2 = _fold_bn(g2, b2, m2, v2)
    bn = np.ascontiguousarray(np.stack([s1, t1, s2, t2]))  # [4, 2, 128]
    x = np.ascontiguousarray(x.astype(np.float32, copy=False))
    return [{
        "x": x[c * NPER:(c + 1) * NPER],
        "w1b": w1b, "w2b": w2b, "bn": bn,
    } for c in range(N_CORES)]


def kernel(x, w1, g1, b1, m1, v1, w2, g2, b2, m2, v2):
    nc = _get_nc()
    in_maps = _prep_in_maps(x, w1, g1, b1, m1, v1, w2, g2, b2, m2, v2)
    res = run_bass_kernel_spmd(nc, in_maps, list(range(N_CORES)))
    out = np.concatenate([res.results[c]["out"] for c in range(N_CORES)], axis=0)
    return out


# revision 23
# speedup vs baseline: 1.1118x; 1.0028x over previous
"""Binarized BasicBlock (BNN) forward on 8 Trainium2 NeuronCores.

Reference computation (per reference.py):
    xb  = sign(x);  wb = sign(w)
    y1  = conv3x3(xb, wb1, pad=1)
    a1  = hardtanh(bn1(y1))          # only sign(a1) feeds conv2
    y2  = conv3x3(sign(a1), wb2, pad=1)
    out = hardtanh(bn2(y2) + x)

Strategy:
  - Data parallel: batch N=64 -> 8 images per core; weights/BN replicated.
  - Conv as 9 shifted matmuls over a zero-padded 58x58 image held in SBUF,
    contraction over input channels: 256 channels = 2 planes of 128
    partitions contracted in ONE matmul via fp8 DoubleRow perf mode.
  - Binarized operands stored as fp8e4 (+-1, 0 exact); PSUM accumulates
    fp32; sums of +-1 with <=2304 terms are exact integers in fp32.
  - BN folded into the activation op: sign(bn1(y)) = Sign(y*s1 + t1) with
    s1 = g1/sqrt(v1+eps), t1 = b1 - m1*s1 (host-folded, passed as inputs).
  - Final stage: Identity(y2*s2+t2) on ScalarE, then add-residual and
    clip (tensor_scalar min+max) on VectorE.

Schedule (the kernel is PE-bound at the fp8 DoubleRow ceiling of 1 px/cyc;
all that matters beyond the 381us matmul floor is head/tail latency):
  - Only sync (SP) and scalar (Activation) queues are hardware DGEs; the
    gpsimd queue is a software DGE with ~5us transfer latency and a slow
    final drain, so it only carries w2 + BN params (needed late, done
    early).
  - Image-0 input is split into 4 row-pieces per 128-ch plane on the sync
    queue so the first conv chunk's rows arrive ASAP; w1 loads
    concurrently on the scalar queue.
  - A short PE warm-up (junk matmuls) covers the input-DMA latency so the
    HAM is ramped when the first real matmul issues.
  - Pad cells of the double-buffered padded-image tiles are zeroed ONCE at
    the head on the vector engine (sign writes never touch them), not per
    image.
  - Outputs stored per 8-row chunk on the sync queue; the last chunk is
    split 6+2 rows so the final store's epilogue chain
    (identity -> add -> clip -> DMA) is short.
"""

import sys

try:
    import concourse  # noqa: F401
except ImportError:  # pragma: no cover
    sys.path.insert(0, "/opt/trn_rl_repo")

import numpy as np
import ml_dtypes

import concourse.bacc as bacc
import concourse.tile as tile
import concourse.mybir as mybir
from concourse.bass_utils import run_bass_kernel_spmd

dt = mybir.dt
AF = mybir.ActivationFunctionType
ALU = mybir.AluOpType
PM = mybir.MatmulPerfMode

N_CORES = 8
NPER = 8          # images per core
C = 256
H = W = 56
HW = H * W        # 3136
WP = 58           # padded row width (1 + 56 + 1)
ROWW = 64         # allocated width per (row, k-plane) block (16B aligned)
RPITCH = 2 * ROWW  # 128 = row pitch (both k-planes interleaved per row)
PROWS = 58        # padded rows
PLSZ = PROWS * RPITCH  # 7424 = padded image tile length
RPC = 8           # output rows per matmul chunk
CHU = RPC * W     # 448 = useful matmul free dim (garbage cols skipped)
NCH = H // RPC    # 7 chunks per image
BN_EPS = 1e-5
WARMUP = 18       # junk matmuls to ramp the PE while the first input loads
# conv2 output-row chunking: 6x8 rows + 6 + 2 (short tail chunk)
CH2 = ((0, 8), (8, 8), (16, 8), (24, 8), (32, 8), (40, 8), (48, 6), (54, 2))
CH1 = tuple((RPC * s, RPC) for s in range(NCH))
# center tap first: it always covers the full chunk, so it carries the
# PSUM-reset start flag; the edge taps may be truncated (pad skipping)
TAP_ORDER = (4, 0, 1, 2, 3, 5, 6, 7, 8)

_CACHE = {}


def _zero_pads(nc, t):
    """Zero the padding cells of a [128, PLSZ] row-interleaved image tile.

    Layout: element (row r, k-plane k, col c) at r*RPITCH + k*ROWW + c;
    c=1..56 hold image cols 0..55, c=0 and c=57..63 are zero pads, rows
    0 and 57 are zero pad rows."""
    v = t[:]
    nc.vector.memset(v[:, 0:RPITCH], 0.0)                      # top pad row
    nc.vector.memset(v[:, 57 * RPITCH:PLSZ], 0.0)              # bottom pad row
    # per-block right pads c=57..63 plus the following block's c=0
    cols = v[:, 57:57 + 57 * RPITCH].rearrange("p (r k c) -> p r k c", k=2, c=ROWW)
    nc.vector.memset(cols[:, :, :, 0:8], 0.0)


def _rview(t):
    # [128, PROWS, 2, ROWW]
    return t[:].rearrange("p (r k c) -> p r k c", k=2, c=ROWW)


def _conv_chunk(nc, ps_t, wt, src_v, ro, nr, co):
    """9-tap conv accumulation for output rows [ro, ro+nr).

    Pad rows/cols contribute zero, so the edge taps skip them instead of
    streaming them through the PE: the dv=0/2 taps drop the top/bottom pad
    row at the image border, the dh=0/2 taps drop the left/right pad
    column everywhere (~2% fewer PE cycles). kk=4 always covers the whole
    chunk and goes first with the PSUM-reset start flag."""
    psv = ps_t[:, 0:nr * W].rearrange("p (r c) -> p r c", c=W)
    for i, kk in enumerate(TAP_ORDER):
        dv, dh = kk // 3, kk % 3
        r_lo, r_hi = ro + dv, ro + dv + nr
        j_lo, j_hi = 0, nr
        if r_lo == 0:
            r_lo, j_lo = 1, 1
        if r_hi == PROWS:
            r_hi, j_hi = PROWS - 1, nr - 1
        c_lo, cw, oc = dh, W, 0
        if dh == 0:
            c_lo, cw, oc = 1, W - 1, 1
        elif dh == 2:
            cw = W - 1
        rhs = src_v[:, r_lo:r_hi, :, c_lo:c_lo + cw].rearrange("p r k c -> p k r c")
        nc.tensor.matmul(
            psv[:, j_lo:j_hi, oc:oc + cw],
            wt[:, :, kk, co * 128:(co + 1) * 128],
            rhs,
            start=(i == 0),
            stop=(i == 8),
            perf_mode=PM.DoubleRow,
        )


def _build():
    nc = bacc.Bacc("TRN2", target_bir_lowering=False, debug=False)

    x_d = nc.dram_tensor("x", [NPER, C, H, W], dt.float32, kind="ExternalInput").ap()
    w1_d = nc.dram_tensor("w1b", [2, 128, 9, C], dt.float8e4, kind="ExternalInput").ap()
    w2_d = nc.dram_tensor("w2b", [2, 128, 9, C], dt.float8e4, kind="ExternalInput").ap()
    # folded BN params packed [v, q, p] with v: 0=s1, 1=t1, 2=s2, 3=t2
    bn_d = nc.dram_tensor("bn", [4, 2, 128], dt.float32, kind="ExternalInput").ap()
    out_d = nc.dram_tensor("out", [NPER, C, H, W], dt.float32, kind="ExternalOutput").ap()

    with tile.TileContext(nc) as tc:
        with (
            tc.tile_pool(name="wp", bufs=1) as wp,
            tc.tile_pool(name="xin", bufs=4) as xinp,
            tc.tile_pool(name="ost", bufs=3) as ostp,
            tc.tile_pool(name="tmp", bufs=4) as tmpp,
            tc.tile_pool(name="ps", bufs=7, space="PSUM") as psp,
            nc.sbuf_tensor([128, PLSZ], dt.float8e4) as xb0,
            nc.sbuf_tensor([128, PLSZ], dt.float8e4) as xb1,
            nc.sbuf_tensor([128, PLSZ], dt.float8e4) as ab0,
            nc.sbuf_tensor([128, PLSZ], dt.float8e4) as ab1,
            nc.sbuf_tensor([128, 2 * CHU], dt.float8e4) as warm_in,
            nc.psum_tensor([128, CHU], dt.float32) as warm_ps,
        ):
            # ---- PE warm-up: junk matmuls on scratch data so the PE is
            # ramped (HAM) when the first real matmul issues; they overlap
            # the first input pieces' DMA+binarize latency. The scratch is
            # read uninitialized on purpose — results are discarded and a
            # memset would delay the first warm-up matmul.
            wv = warm_in[:].rearrange("p (k c) -> p k c", k=2)
            for _ in range(WARMUP):
                nc.tensor.matmul(
                    warm_ps[:], wv[:, :, 0:128], wv[:],
                    start=True, stop=True, perf_mode=PM.DoubleRow,
                )

            # Pad cells are read-only for matmuls and never overwritten by
            # the sign writes (cols 1..56 only): zero them once per buffer.
            # Only xb0's pads gate the first matmul; the other three tiles'
            # pads are emitted after image 0's binarize (vector engine order)
            # since they are not read until image 0's conv2 / image 1.
            _zero_pads(nc, xb0)

            # w1 heads the scalar queue (the first real matmul needs it);
            # image-0's last two q1 pieces share that queue. w2 + BN ride
            # the gpsimd software-DGE queue (high latency, needed late).
            w_sb = []
            for wd, tag in ((w1_d, "w1"), (w2_d, "w2")):
                t = wp.tile([128, 2, 9, C], dt.float8e4, tag=tag)
                w_sb.append(t)
            bn_sb = wp.tile([128, 8], dt.float32, tag="bn")
            nc.scalar.dma_start(w_sb[0][:], w1_d.rearrange("q p k c -> p q k c"))
            nc.gpsimd.dma_start(bn_sb[:], bn_d.rearrange("v q p -> p (v q)"))
            nc.gpsimd.dma_start(w_sb[1][:], w2_d.rearrange("q p k c -> p q k c"))

            for n in range(NPER):
                xb = (xb0, xb1)[n % 2]
                ab = (ab0, ab1)[n % 2]
                xbv = _rview(xb)
                abv = _rview(ab)

                # ---- load + binarize input ----
                xin = []
                for q in range(2):
                    xi = xinp.tile([128, HW], dt.float32, tag="xin")
                    xin.append(xi)

                def _load(q, r0, nr, dma_eng):
                    dma_eng.dma_start(
                        xin[q][:, r0 * W:(r0 + nr) * W],
                        x_d[n, q * 128:(q + 1) * 128, r0:r0 + nr].rearrange(
                            "p h w -> p (h w)"),
                    )

                def _sign_scalar(q, r0, nr):
                    nc.scalar.activation(
                        xbv[:, 1 + r0:1 + r0 + nr, q, 1:57],
                        xin[q][:, r0 * W:(r0 + nr) * W].rearrange(
                            "p (h w) -> p h w", w=W),
                        AF.Sign,
                    )

                def _sign_vector(q, r0, nr):
                    # sign() emulated as (x > 0) * 2 - 1 so the binarize can
                    # run off the scalar engine during the kernel head
                    dst = xbv[:, 1 + r0:1 + r0 + nr, q, 1:57]
                    src = xin[q][:, r0 * W:(r0 + nr) * W].rearrange(
                        "p (h w) -> p h w", w=W)
                    nc.vector.tensor_scalar(dst, src, 0.0, None, ALU.is_gt)
                    nc.vector.tensor_scalar(dst, dst, 2.0, -1.0, ALU.mult, ALU.add)

                if n == 0:
                    # Image 0 is the kernel head, and it is HBM-bandwidth
                    # bound (~11us of wire time for image + w1): stream the
                    # input in row order, q0/q1 interleaved on the sync
                    # queue, so conv1's chunks unlock as rows arrive. The
                    # q1 planes binarize on the vector engine to halve the
                    # scalar engine's head-critical sign latency.
                    pieces = ((0, 10), (10, 12), (22, 12), (34, 10), (44, 12))
                    for r0, nr in pieces:
                        _load(0, r0, nr, nc.sync)
                        _load(1, r0, nr, nc.sync)
                    for r0, nr in pieces:
                        _sign_scalar(0, r0, nr)
                        _sign_vector(1, r0, nr)
                    for t in (ab0, xb1, ab1):
                        _zero_pads(nc, t)
                else:
                    for r0, nr in ((0, 28), (28, 28)):
                        for q in range(2):
                            _load(q, r0, nr, nc.sync)
                            _sign_scalar(q, r0, nr)

                # ---- conv1 -> sign(bn1(.)) into padded intermediate ----
                # chunk-outer / co-inner: during the head, each input piece
                # unlocks both co passes of a chunk (2x the PE work per
                # piece), which hides the input stream's arrival pace
                for ro, nr in CH1:
                    for co in range(2):
                        ps = psp.tile([128, CHU], dt.float32, tag="ps")
                        _conv_chunk(nc, ps, w_sb[0], xbv, ro, nr, co)
                        psv = ps[:, 0:nr * W].rearrange("p (r c) -> p r c", c=W)
                        nc.scalar.activation(
                            abv[:, 1 + ro:1 + ro + nr, co, 1:57], psv, AF.Sign,
                            bias=bn_sb[:, 2 + co:3 + co], scale=bn_sb[:, 0 + co:1 + co],
                        )

                # ---- conv2 -> bn2 + residual + clip -> store per chunk ----
                for co in range(2):
                    ost = ostp.tile([128, HW], dt.float32, tag="ost")
                    ostv = ost[:].rearrange("p (h w) -> p h w", w=W)
                    xinv = xin[co][:].rearrange("p (h w) -> p h w", w=W)
                    for ro, nr in CH2:
                        fd = nr * W
                        pst = psp.tile([128, CHU], dt.float32, tag="ps")
                        _conv_chunk(nc, pst, w_sb[1], abv, ro, nr, co)
                        psv = pst[:, 0:fd].rearrange("p (r c) -> p r c", c=W)
                        tm = tmpp.tile([128, CHU], dt.float32, tag="tmp")
                        tmv = tm[:, 0:fd].rearrange("p (r c) -> p r c", c=W)
                        nc.scalar.activation(
                            tmv, psv, AF.Identity,
                            bias=bn_sb[:, 6 + co:7 + co], scale=bn_sb[:, 4 + co:5 + co],
                        )
                        ov = ostv[:, ro:ro + nr, :]
                        nc.vector.tensor_tensor(
                            ov, tmv, xinv[:, ro:ro + nr, :], ALU.add
                        )
                        nc.vector.tensor_scalar(ov, ov, 1.0, -1.0, ALU.min, ALU.max)
                        # last chunk's store rides the scalar queue: at the
                        # kernel tail the sync engine lags on earlier stores
                        st_eng = nc.scalar if ro == 54 else nc.sync
                        st_eng.dma_start(
                            out_d[n, co * 128:(co + 1) * 128, ro:ro + nr].rearrange(
                                "p h w -> p (h w)"),
                            ost[:, ro * W:(ro + nr) * W],
                        )

    nc.compile()
    return nc


def _get_nc():
    if "nc" not in _CACHE:
        _CACHE["nc"] = _build()
    return _CACHE["nc"]


def _prep_weights(w):
    # [co, cin, kh, kw] -> [cin_chunk 2, cin 128, tap 9, co 256], binarized fp8e4
    a = np.sign(w.astype(np.float32))
    a = a.transpose(1, 2, 3, 0).reshape(2, 128, 9, C)
    return np.ascontiguousarray(a.astype(ml_dtypes.float8_e4m3))


def _fold_bn(g, b, m, v):
    s = (g.astype(np.float32) / np.sqrt(v.astype(np.float32) + BN_EPS)).astype(np.float32)
    t = (b.astype(np.float32) - m.astype(np.float32) * s).astype(np.float32)
    return s.reshape(2, 128), t.reshape(2, 128)


def _prep_in_maps(x, w1, g1, b1, m1, v1, w2, g2, b2, m2, v2):
    w1b = _prep_weights(w1)
    w2b = _prep_weights(w2)
    s1, t1 = _fold_bn(g1, b1, m1, v1)
    s2, t2 = _fold_bn(g2, b2, m2, v2)
    bn = np.ascontiguousarray(np.stack([s1, t1, s2, t2]))  # [4, 2, 128]
    x = np.ascontiguousarray(x.astype(np.float32, copy=False))
    return [{
        "x": x[c * NPER:(c + 1) * NPER],
        "w1b": w1b, "w2b": w2b, "bn": bn,
    } for c in range(N_CORES)]


def kernel(x, w1, g1, b1, m1, v1, w2, g2, b2, m2, v2):
    nc = _get_nc()
    in_maps = _prep_in_maps(x, w1, g1, b1, m1, v1, w2, g2, b2, m2, v2)
    res = run_bass_kernel_spmd(nc, in_maps, list(range(N_CORES)))
    out = np.concatenate([res.results[c]["out"] for c in range(N_CORES)], axis=0)
    return out


# revision 25
# speedup vs baseline: 1.1137x; 1.0017x over previous
"""Binarized BasicBlock (BNN) forward on 8 Trainium2 NeuronCores.

Reference computation (per reference.py):
    xb  = sign(x);  wb = sign(w)
    y1  = conv3x3(xb, wb1, pad=1)
    a1  = hardtanh(bn1(y1))          # only sign(a1) feeds conv2
    y2  = conv3x3(sign(a1), wb2, pad=1)
    out = hardtanh(bn2(y2) + x)

Strategy:
  - Data parallel: batch N=64 -> 8 images per core; weights/BN replicated.
  - Conv as 9 shifted matmuls over a zero-padded 58x58 image held in SBUF,
    contraction over input channels: 256 channels = 2 planes of 128
    partitions contracted in ONE matmul via fp8 DoubleRow perf mode.
  - Binarized operands stored as fp8e4 (+-1, 0 exact); PSUM accumulates
    fp32; sums of +-1 with <=2304 terms are exact integers in fp32.
  - BN folded into the activation op: sign(bn1(y)) = Sign(y*s1 + t1) with
    s1 = g1/sqrt(v1+eps), t1 = b1 - m1*s1 (host-folded, passed as inputs).
  - Final stage: Identity(y2*s2+t2) on ScalarE, then add-residual and
    clip (tensor_scalar min+max) on VectorE.

Schedule (the kernel is PE-bound at the fp8 DoubleRow ceiling of 1 px/cyc;
all that matters beyond the 381us matmul floor is head/tail latency):
  - Only sync (SP) and scalar (Activation) queues are hardware DGEs; the
    gpsimd queue is a software DGE with ~5us transfer latency and a slow
    final drain, so it only carries w2 + BN params (needed late, done
    early).
  - Image-0 input is split into 4 row-pieces per 128-ch plane on the sync
    queue so the first conv chunk's rows arrive ASAP; w1 loads
    concurrently on the scalar queue.
  - A short PE warm-up (junk matmuls) covers the input-DMA latency so the
    HAM is ramped when the first real matmul issues.
  - Pad cells of the double-buffered padded-image tiles are zeroed ONCE at
    the head on the vector engine (sign writes never touch them), not per
    image.
  - Outputs stored per 8-row chunk on the sync queue; the last chunk is
    split 6+2 rows so the final store's epilogue chain
    (identity -> add -> clip -> DMA) is short.
"""

import sys

try:
    import concourse  # noqa: F401
except ImportError:  # pragma: no cover
    sys.path.insert(0, "/opt/trn_rl_repo")

import numpy as np
import ml_dtypes

import concourse.bacc as bacc
import concourse.tile as tile
import concourse.mybir as mybir
from concourse.bass_utils import run_bass_kernel_spmd

dt = mybir.dt
AF = mybir.ActivationFunctionType
ALU = mybir.AluOpType
PM = mybir.MatmulPerfMode

N_CORES = 8
NPER = 8          # images per core
C = 256
H = W = 56
HW = H * W        # 3136
WP = 58           # padded row width (1 + 56 + 1)
ROWW = 64         # allocated width per (row, k-plane) block (16B aligned)
RPITCH = 2 * ROWW  # 128 = row pitch (both k-planes interleaved per row)
PROWS = 58        # padded rows
PLSZ = PROWS * RPITCH  # 7424 = padded image tile length
RPC = 8           # output rows per matmul chunk
CHU = RPC * W     # 448 = useful matmul free dim (garbage cols skipped)
NCH = H // RPC    # 7 chunks per image
BN_EPS = 1e-5
WARMUP = 28       # junk matmuls to ramp the PE while the first input loads
# conv2 output-row chunking: 6x8 rows + 6 + 2 (short tail chunk)
CH2 = ((0, 8), (8, 8), (16, 8), (24, 8), (32, 8), (40, 8), (48, 6), (54, 2))
CH1 = tuple((RPC * s, RPC) for s in range(NCH))
# center tap first: it always covers the full chunk, so it carries the
# PSUM-reset start flag; the edge taps may be truncated (pad skipping)
TAP_ORDER = (4, 0, 1, 2, 3, 5, 6, 7, 8)

_CACHE = {}


def _zero_pads(nc, t):
    """Zero the padding cells of a [128, PLSZ] row-interleaved image tile.

    Layout: element (row r, k-plane k, col c) at r*RPITCH + k*ROWW + c;
    c=1..56 hold image cols 0..55, c=0 and c=57..63 are zero pads, rows
    0 and 57 are zero pad rows."""
    v = t[:]
    nc.vector.memset(v[:, 0:RPITCH], 0.0)                      # top pad row
    nc.vector.memset(v[:, 57 * RPITCH:PLSZ], 0.0)              # bottom pad row
    # per-block right pads c=57..63 plus the following block's c=0
    cols = v[:, 57:57 + 57 * RPITCH].rearrange("p (r k c) -> p r k c", k=2, c=ROWW)
    nc.vector.memset(cols[:, :, :, 0:8], 0.0)


def _rview(t):
    # [128, PROWS, 2, ROWW]
    return t[:].rearrange("p (r k c) -> p r k c", k=2, c=ROWW)


def _conv_chunk(nc, ps_t, wt, src_v, ro, nr, co):
    """9-tap conv accumulation for output rows [ro, ro+nr).

    Pad rows/cols contribute zero, so the edge taps skip them instead of
    streaming them through the PE: the dv=0/2 taps drop the top/bottom pad
    row at the image border, the dh=0/2 taps drop the left/right pad
    column everywhere (~2% fewer PE cycles). kk=4 always covers the whole
    chunk and goes first with the PSUM-reset start flag."""
    psv = ps_t[:, 0:nr * W].rearrange("p (r c) -> p r c", c=W)
    for i, kk in enumerate(TAP_ORDER):
        dv, dh = kk // 3, kk % 3
        r_lo, r_hi = ro + dv, ro + dv + nr
        j_lo, j_hi = 0, nr
        if r_lo == 0:
            r_lo, j_lo = 1, 1
        if r_hi == PROWS:
            r_hi, j_hi = PROWS - 1, nr - 1
        c_lo, cw, oc = dh, W, 0
        if dh == 0:
            c_lo, cw, oc = 1, W - 1, 1
        elif dh == 2:
            cw = W - 1
        rhs = src_v[:, r_lo:r_hi, :, c_lo:c_lo + cw].rearrange("p r k c -> p k r c")
        nc.tensor.matmul(
            psv[:, j_lo:j_hi, oc:oc + cw],
            wt[:, :, kk, co * 128:(co + 1) * 128],
            rhs,
            start=(i == 0),
            stop=(i == 8),
            perf_mode=PM.DoubleRow,
        )


def _build():
    nc = bacc.Bacc("TRN2", target_bir_lowering=False, debug=False)

    x_d = nc.dram_tensor("x", [NPER, C, H, W], dt.float32, kind="ExternalInput").ap()
    w1_d = nc.dram_tensor("w1b", [2, 128, 9, C], dt.float8e4, kind="ExternalInput").ap()
    w2_d = nc.dram_tensor("w2b", [2, 128, 9, C], dt.float8e4, kind="ExternalInput").ap()
    # folded BN params packed [v, q, p] with v: 0=s1, 1=t1, 2=s2, 3=t2
    bn_d = nc.dram_tensor("bn", [4, 2, 128], dt.float32, kind="ExternalInput").ap()
    out_d = nc.dram_tensor("out", [NPER, C, H, W], dt.float32, kind="ExternalOutput").ap()

    with tile.TileContext(nc) as tc:
        with (
            tc.tile_pool(name="wp", bufs=1) as wp,
            tc.tile_pool(name="xin", bufs=4) as xinp,
            tc.tile_pool(name="ost", bufs=3) as ostp,
            tc.tile_pool(name="tmp", bufs=4) as tmpp,
            tc.tile_pool(name="ps", bufs=7, space="PSUM") as psp,
            nc.sbuf_tensor([128, PLSZ], dt.float8e4) as xb0,
            nc.sbuf_tensor([128, PLSZ], dt.float8e4) as xb1,
            nc.sbuf_tensor([128, PLSZ], dt.float8e4) as ab0,
            nc.sbuf_tensor([128, PLSZ], dt.float8e4) as ab1,
            nc.sbuf_tensor([128, 2 * CHU], dt.float8e4) as warm_in,
            nc.psum_tensor([128, CHU], dt.float32) as warm_ps,
        ):
            # ---- PE warm-up: junk matmuls on scratch data so the PE is
            # ramped (HAM) when the first real matmul issues; they overlap
            # the first input pieces' DMA+binarize latency. The scratch is
            # read uninitialized on purpose — results are discarded and a
            # memset would delay the first warm-up matmul.
            wv = warm_in[:].rearrange("p (k c) -> p k c", k=2)
            for _ in range(WARMUP):
                nc.tensor.matmul(
                    warm_ps[:], wv[:, :, 0:128], wv[:],
                    start=True, stop=True, perf_mode=PM.DoubleRow,
                )

            # Pad cells are read-only for matmuls and never overwritten by
            # the sign writes (cols 1..56 only): zero them once per buffer.
            # Only xb0's pads gate the first matmul; the other three tiles'
            # pads are emitted after image 0's binarize (vector engine order)
            # since they are not read until image 0's conv2 / image 1.
            _zero_pads(nc, xb0)

            # w1 heads the scalar queue (the first real matmul needs it);
            # image-0's last two q1 pieces share that queue. w2 + BN ride
            # the gpsimd software-DGE queue (high latency, needed late).
            w_sb = []
            for wd, tag in ((w1_d, "w1"), (w2_d, "w2")):
                t = wp.tile([128, 2, 9, C], dt.float8e4, tag=tag)
                w_sb.append(t)
            bn_sb = wp.tile([128, 8], dt.float32, tag="bn")
            nc.scalar.dma_start(w_sb[0][:], w1_d.rearrange("q p k c -> p q k c"))
            nc.gpsimd.dma_start(bn_sb[:], bn_d.rearrange("v q p -> p (v q)"))
            nc.gpsimd.dma_start(w_sb[1][:], w2_d.rearrange("q p k c -> p q k c"))

            for n in range(NPER):
                xb = (xb0, xb1)[n % 2]
                ab = (ab0, ab1)[n % 2]
                xbv = _rview(xb)
                abv = _rview(ab)

                # ---- load + binarize input ----
                xin = []
                for q in range(2):
                    xi = xinp.tile([128, HW], dt.float32, tag="xin")
                    xin.append(xi)

                def _load(q, r0, nr, dma_eng):
                    dma_eng.dma_start(
                        xin[q][:, r0 * W:(r0 + nr) * W],
                        x_d[n, q * 128:(q + 1) * 128, r0:r0 + nr].rearrange(
                            "p h w -> p (h w)"),
                    )

                def _sign_scalar(q, r0, nr):
                    nc.scalar.activation(
                        xbv[:, 1 + r0:1 + r0 + nr, q, 1:57],
                        xin[q][:, r0 * W:(r0 + nr) * W].rearrange(
                            "p (h w) -> p h w", w=W),
                        AF.Sign,
                    )

                def _sign_vector(q, r0, nr):
                    # sign() emulated as (x > 0) * 2 - 1 so the binarize can
                    # run off the scalar engine during the kernel head
                    dst = xbv[:, 1 + r0:1 + r0 + nr, q, 1:57]
                    src = xin[q][:, r0 * W:(r0 + nr) * W].rearrange(
                        "p (h w) -> p h w", w=W)
                    nc.vector.tensor_scalar(dst, src, 0.0, None, ALU.is_gt)
                    nc.vector.tensor_scalar(dst, dst, 2.0, -1.0, ALU.mult, ALU.add)

                if n == 0:
                    # Image 0 is the kernel head, and it is HBM-bandwidth
                    # bound (~11us of wire time for image + w1): stream the
                    # input in row order, q0/q1 interleaved on the sync
                    # queue, so conv1's chunks unlock as rows arrive. The
                    # q1 planes binarize on the vector engine to halve the
                    # scalar engine's head-critical sign latency.
                    pieces = ((0, 9), (9, 9), (18, 14), (32, 12), (44, 12))
                    for r0, nr in pieces:
                        _load(0, r0, nr, nc.sync)
                        _load(1, r0, nr, nc.sync)
                    for r0, nr in pieces:
                        _sign_scalar(0, r0, nr)
                        _sign_vector(1, r0, nr)
                    for t in (ab0, xb1, ab1):
                        _zero_pads(nc, t)
                else:
                    for r0, nr in ((0, 28), (28, 28)):
                        for q in range(2):
                            _load(q, r0, nr, nc.sync)
                            _sign_scalar(q, r0, nr)

                # ---- conv1 -> sign(bn1(.)) into padded intermediate ----
                # chunk-outer / co-inner: during the head, each input piece
                # unlocks both co passes of a chunk (2x the PE work per
                # piece), which hides the input stream's arrival pace
                for ro, nr in CH1:
                    for co in range(2):
                        ps = psp.tile([128, CHU], dt.float32, tag="ps")
                        _conv_chunk(nc, ps, w_sb[0], xbv, ro, nr, co)
                        psv = ps[:, 0:nr * W].rearrange("p (r c) -> p r c", c=W)
                        nc.scalar.activation(
                            abv[:, 1 + ro:1 + ro + nr, co, 1:57], psv, AF.Sign,
                            bias=bn_sb[:, 2 + co:3 + co], scale=bn_sb[:, 0 + co:1 + co],
                        )

                # ---- conv2 -> bn2 + residual + clip -> store per chunk ----
                for co in range(2):
                    ost = ostp.tile([128, HW], dt.float32, tag="ost")
                    ostv = ost[:].rearrange("p (h w) -> p h w", w=W)
                    xinv = xin[co][:].rearrange("p (h w) -> p h w", w=W)
                    for ro, nr in CH2:
                        fd = nr * W
                        pst = psp.tile([128, CHU], dt.float32, tag="ps")
                        _conv_chunk(nc, pst, w_sb[1], abv, ro, nr, co)
                        psv = pst[:, 0:fd].rearrange("p (r c) -> p r c", c=W)
                        tm = tmpp.tile([128, CHU], dt.float32, tag="tmp")
                        tmv = tm[:, 0:fd].rearrange("p (r c) -> p r c", c=W)
                        nc.scalar.activation(
                            tmv, psv, AF.Identity,
                            bias=bn_sb[:, 6 + co:7 + co], scale=bn_sb[:, 4 + co:5 + co],
                        )
                        ov = ostv[:, ro:ro + nr, :]
                        nc.vector.tensor_tensor(
                            ov, tmv, xinv[:, ro:ro + nr, :], ALU.add
                        )
                        nc.vector.tensor_scalar(ov, ov, 1.0, -1.0, ALU.min, ALU.max)
                        # last chunk's store rides the scalar queue: at the
                        # kernel tail the sync engine lags on earlier stores
                        st_eng = nc.scalar if ro == 54 else nc.sync
                        st_eng.dma_start(
                            out_d[n, co * 128:(co + 1) * 128, ro:ro + nr].rearrange(
                                "p h w -> p (h w)"),
                            ost[:, ro * W:(ro + nr) * W],
                        )

    nc.compile()
    return nc


def _get_nc():
    if "nc" not in _CACHE:
        _CACHE["nc"] = _build()
    return _CACHE["nc"]


def _prep_weights(w):
    # [co, cin, kh, kw] -> [cin_chunk 2, cin 128, tap 9, co 256], binarized fp8e4
    a = np.sign(w.astype(np.float32))
    a = a.transpose(1, 2, 3, 0).reshape(2, 128, 9, C)
    return np.ascontiguousarray(a.astype(ml_dtypes.float8_e4m3))


def _fold_bn(g, b, m, v):
    s = (g.astype(np.float32) / np.sqrt(v.astype(np.float32) + BN_EPS)).astype(np.float32)
    t = (b.astype(np.float32) - m.astype(np.float32) * s).astype(np.float32)
    return s.reshape(2, 128), t.reshape(2, 128)


def _prep_in_maps(x, w1, g1, b1, m1, v1, w2, g2, b2, m2, v2):
    w1b = _prep_weights(w1)
    w2b = _prep_weights(w2)
    s1, t1 = _fold_bn(g1, b1, m1, v1)
    s2, t2 = _fold_bn(g2, b2, m2, v2)
    bn = np.ascontiguousarray(np.stack([s1, t1, s2, t2]))  # [4, 2, 128]
    x = np.ascontiguousarray(x.astype(np.float32, copy=False))
    return [{
        "x": x[c * NPER:(c + 1) * NPER],
        "w1b": w1b, "w2b": w2b, "bn": bn,
    } for c in range(N_CORES)]


def kernel(x, w1, g1, b1, m1, v1, w2, g2, b2, m2, v2):
    nc = _get_nc()
    in_maps = _prep_in_maps(x, w1, g1, b1, m1, v1, w2, g2, b2, m2, v2)
    res = run_bass_kernel_spmd(nc, in_maps, list(range(N_CORES)))
    out = np.concatenate([res.results[c]["out"] for c in range(N_CORES)], axis=0)
    return out


# revision 28
# speedup vs baseline: 1.1159x; 1.0020x over previous
"""Binarized BasicBlock (BNN) forward on 8 Trainium2 NeuronCores.

Reference computation (per reference.py):
    xb  = sign(x);  wb = sign(w)
    y1  = conv3x3(xb, wb1, pad=1)
    a1  = hardtanh(bn1(y1))          # only sign(a1) feeds conv2
    y2  = conv3x3(sign(a1), wb2, pad=1)
    out = hardtanh(bn2(y2) + x)

Strategy:
  - Data parallel: batch N=64 -> 8 images per core; weights/BN replicated.
  - Conv as 9 shifted matmuls over a zero-padded 58x58 image held in SBUF,
    contraction over input channels: 256 channels = 2 planes of 128
    partitions contracted in ONE matmul via fp8 DoubleRow perf mode.
  - Binarized operands stored as fp8e4 (+-1, 0 exact); PSUM accumulates
    fp32; sums of +-1 with <=2304 terms are exact integers in fp32.
  - BN folded into the activation op: sign(bn1(y)) = Sign(y*s1 + t1) with
    s1 = g1/sqrt(v1+eps), t1 = b1 - m1*s1 (host-folded, passed as inputs).
  - Final stage: Identity(y2*s2+t2) on ScalarE, then add-residual and
    clip (tensor_scalar min+max) on VectorE.

Schedule (the kernel is PE-bound at the fp8 DoubleRow ceiling of 1 px/cyc;
all that matters beyond the 381us matmul floor is head/tail latency):
  - Only sync (SP) and scalar (Activation) queues are hardware DGEs; the
    gpsimd queue is a software DGE with ~5us transfer latency and a slow
    final drain, so it only carries w2 + BN params (needed late, done
    early).
  - Image-0 input is split into 4 row-pieces per 128-ch plane on the sync
    queue so the first conv chunk's rows arrive ASAP; w1 loads
    concurrently on the scalar queue.
  - A short PE warm-up (junk matmuls) covers the input-DMA latency so the
    HAM is ramped when the first real matmul issues.
  - Pad cells of the double-buffered padded-image tiles are zeroed ONCE at
    the head on the vector engine (sign writes never touch them), not per
    image.
  - Outputs stored per 8-row chunk on the sync queue; the last chunk is
    split 6+2 rows so the final store's epilogue chain
    (identity -> add -> clip -> DMA) is short.
"""

import sys

try:
    import concourse  # noqa: F401
except ImportError:  # pragma: no cover
    sys.path.insert(0, "/opt/trn_rl_repo")

import numpy as np
import ml_dtypes

import concourse.bacc as bacc
import concourse.tile as tile
import concourse.mybir as mybir
from concourse.bass_utils import run_bass_kernel_spmd

dt = mybir.dt
AF = mybir.ActivationFunctionType
ALU = mybir.AluOpType
PM = mybir.MatmulPerfMode

N_CORES = 8
NPER = 8          # images per core
C = 256
H = W = 56
HW = H * W        # 3136
WP = 58           # padded row width (1 + 56 + 1)
ROWW = 64         # allocated width per (row, k-plane) block (16B aligned)
RPITCH = 2 * ROWW  # 128 = row pitch (both k-planes interleaved per row)
PROWS = 58        # padded rows
PLSZ = PROWS * RPITCH  # 7424 = padded image tile length
RPC = 8           # output rows per matmul chunk
CHU = RPC * W     # 448 = useful matmul free dim (garbage cols skipped)
NCH = H // RPC    # 7 chunks per image
BN_EPS = 1e-5
WARMUP = 28       # junk matmuls to ramp the PE while the first input loads
# conv2 output-row chunking: 6x8 rows + 6 + 2 (short tail chunk)
CH2 = ((0, 8), (8, 8), (16, 8), (24, 8), (32, 8), (40, 8), (48, 6), (54, 2))
CH1 = tuple((RPC * s, RPC) for s in range(NCH))
# center tap first: it always covers the full chunk, so it carries the
# PSUM-reset start flag; the edge taps may be truncated (pad skipping)
TAP_ORDER = (4, 0, 1, 2, 3, 5, 6, 7, 8)

_CACHE = {}


def _zero_pads(nc, t):
    """Zero the padding cells of a [128, PLSZ] row-interleaved image tile.

    Layout: element (row r, k-plane k, col c) at r*RPITCH + k*ROWW + c;
    c=1..56 hold image cols 0..55, c=0 and c=57..63 are zero pads, rows
    0 and 57 are zero pad rows."""
    v = t[:]
    nc.vector.memset(v[:, 0:RPITCH], 0.0)                      # top pad row
    nc.vector.memset(v[:, 57 * RPITCH:PLSZ], 0.0)              # bottom pad row
    # per-block right pads c=57..63 plus the following block's c=0
    cols = v[:, 57:57 + 57 * RPITCH].rearrange("p (r k c) -> p r k c", k=2, c=ROWW)
    nc.vector.memset(cols[:, :, :, 0:8], 0.0)


def _rview(t):
    # [128, PROWS, 2, ROWW]
    return t[:].rearrange("p (r k c) -> p r k c", k=2, c=ROWW)


def _conv_chunk(nc, ps_t, wt, src_v, ro, nr, co):
    """9-tap conv accumulation for output rows [ro, ro+nr).

    Pad rows/cols contribute zero, so the edge taps skip them instead of
    streaming them through the PE: the dv=0/2 taps drop the top/bottom pad
    row at the image border, the dh=0/2 taps drop the left/right pad
    column everywhere (~2% fewer PE cycles). kk=4 always covers the whole
    chunk and goes first with the PSUM-reset start flag."""
    psv = ps_t[:, 0:nr * W].rearrange("p (r c) -> p r c", c=W)
    for i, kk in enumerate(TAP_ORDER):
        dv, dh = kk // 3, kk % 3
        r_lo, r_hi = ro + dv, ro + dv + nr
        j_lo, j_hi = 0, nr
        if r_lo == 0:
            r_lo, j_lo = 1, 1
        if r_hi == PROWS:
            r_hi, j_hi = PROWS - 1, nr - 1
        c_lo, cw, oc = dh, W, 0
        if dh == 0:
            c_lo, cw, oc = 1, W - 1, 1
        elif dh == 2:
            cw = W - 1
        rhs = src_v[:, r_lo:r_hi, :, c_lo:c_lo + cw].rearrange("p r k c -> p k r c")
        nc.tensor.matmul(
            psv[:, j_lo:j_hi, oc:oc + cw],
            wt[:, :, kk, co * 128:(co + 1) * 128],
            rhs,
            start=(i == 0),
            stop=(i == 8),
            perf_mode=PM.DoubleRow,
        )


def _build():
    nc = bacc.Bacc("TRN2", target_bir_lowering=False, debug=False)

    x_d = nc.dram_tensor("x", [NPER, C, H, W], dt.float32, kind="ExternalInput").ap()
    w1_d = nc.dram_tensor("w1b", [2, 128, 9, C], dt.float8e4, kind="ExternalInput").ap()
    w2_d = nc.dram_tensor("w2b", [2, 128, 9, C], dt.float8e4, kind="ExternalInput").ap()
    # folded BN params packed [v, q, p] with v: 0=s1, 1=t1, 2=s2, 3=t2
    bn_d = nc.dram_tensor("bn", [4, 2, 128], dt.float32, kind="ExternalInput").ap()
    out_d = nc.dram_tensor("out", [NPER, C, H, W], dt.float32, kind="ExternalOutput").ap()

    with tile.TileContext(nc) as tc:
        with (
            tc.tile_pool(name="wp", bufs=1) as wp,
            tc.tile_pool(name="xin", bufs=2) as xinp,
            tc.tile_pool(name="ost", bufs=3) as ostp,
            tc.tile_pool(name="tmp", bufs=4) as tmpp,
            tc.tile_pool(name="ps", bufs=7, space="PSUM") as psp,
            nc.sbuf_tensor([128, PLSZ], dt.float8e4) as xb0,
            nc.sbuf_tensor([128, PLSZ], dt.float8e4) as xb1,
            nc.sbuf_tensor([128, PLSZ], dt.float8e4) as ab0,
            nc.sbuf_tensor([128, PLSZ], dt.float8e4) as ab1,
            nc.sbuf_tensor([128, 2 * CHU], dt.float8e4) as warm_in,
            nc.psum_tensor([128, CHU], dt.float32) as warm_ps,
        ):
            # ---- PE warm-up: junk matmuls on scratch data so the PE is
            # ramped (HAM) when the first real matmul issues; they overlap
            # the first input pieces' DMA+binarize latency. The scratch is
            # read uninitialized on purpose — results are discarded and a
            # memset would delay the first warm-up matmul.
            wv = warm_in[:].rearrange("p (k c) -> p k c", k=2)
            for _ in range(WARMUP):
                nc.tensor.matmul(
                    warm_ps[:], wv[:, :, 0:128], wv[:],
                    start=True, stop=True, perf_mode=PM.DoubleRow,
                )

            # Pad cells are read-only for matmuls and never overwritten by
            # the sign writes (cols 1..56 only): zero them once per buffer.
            # Only xb0's pads gate the first matmul; the other three tiles'
            # pads are emitted after image 0's binarize (vector engine order)
            # since they are not read until image 0's conv2 / image 1.
            _zero_pads(nc, xb0)

            # w1 heads the scalar queue (the first real matmul needs it);
            # image-0's last two q1 pieces share that queue. w2 + BN ride
            # the gpsimd software-DGE queue (high latency, needed late).
            w_sb = []
            for wd, tag in ((w1_d, "w1"), (w2_d, "w2")):
                t = wp.tile([128, 2, 9, C], dt.float8e4, tag=tag)
                w_sb.append(t)
            bn_sb = wp.tile([128, 8], dt.float32, tag="bn")
            nc.scalar.dma_start(w_sb[0][:], w1_d.rearrange("q p k c -> p q k c"))
            nc.gpsimd.dma_start(bn_sb[:], bn_d.rearrange("v q p -> p (v q)"))
            nc.gpsimd.dma_start(w_sb[1][:], w2_d.rearrange("q p k c -> p q k c"))

            for n in range(NPER):
                xb = (xb0, xb1)[n % 2]
                ab = (ab0, ab1)[n % 2]
                xbv = _rview(xb)
                abv = _rview(ab)

                # ---- load + binarize input ----
                # one tile holds both 128-channel planes: each row piece
                # loads q0+q1 in a single DMA so both planes arrive together
                xin = xinp.tile([128, 2 * HW], dt.float32, tag="xin")
                xiq = xin[:].rearrange("p (q hw) -> p q hw", q=2)

                def _load(r0, nr, dma_eng):
                    dma_eng.dma_start(
                        xiq[:, :, r0 * W:(r0 + nr) * W],
                        x_d[n, :, r0:r0 + nr].rearrange(
                            "(q p) h w -> p q (h w)", p=128),
                    )

                def _sign_scalar(q, r0, nr):
                    nc.scalar.activation(
                        xbv[:, 1 + r0:1 + r0 + nr, q, 1:57],
                        xiq[:, q, r0 * W:(r0 + nr) * W].rearrange(
                            "p (h w) -> p h w", w=W),
                        AF.Sign,
                    )

                def _sign_vector(q, r0, nr):
                    # sign() emulated as (x > 0) * 2 - 1 so the binarize can
                    # run off the scalar engine during the kernel head
                    dst = xbv[:, 1 + r0:1 + r0 + nr, q, 1:57]
                    src = xiq[:, q, r0 * W:(r0 + nr) * W].rearrange(
                        "p (h w) -> p h w", w=W)
                    nc.vector.tensor_scalar(dst, src, 0.0, None, ALU.is_gt)
                    nc.vector.tensor_scalar(dst, dst, 2.0, -1.0, ALU.mult, ALU.add)

                if n == 0:
                    # Image 0 is the kernel head, and it is HBM-bandwidth
                    # bound (~11us of wire time for image + w1): stream the
                    # input in row order on the sync queue so conv1's chunks
                    # unlock as rows arrive. The q1 planes binarize on the
                    # vector engine, in parallel with the scalar q0 signs.
                    pieces = ((0, 9), (9, 9), (18, 14), (32, 12), (44, 12))
                    for r0, nr in pieces:
                        _load(r0, nr, nc.sync)
                    for r0, nr in pieces:
                        _sign_scalar(0, r0, nr)
                        _sign_vector(1, r0, nr)
                    for t in (ab0, xb1, ab1):
                        _zero_pads(nc, t)
                else:
                    for r0, nr in ((0, 28), (28, 28)):
                        _load(r0, nr, nc.sync)
                        for q in range(2):
                            _sign_scalar(q, r0, nr)

                # ---- conv1 -> sign(bn1(.)) into padded intermediate ----
                # chunk-outer / co-inner: during the head, each input piece
                # unlocks both co passes of a chunk (2x the PE work per
                # piece), which hides the input stream's arrival pace
                for ro, nr in CH1:
                    for co in range(2):
                        ps = psp.tile([128, CHU], dt.float32, tag="ps")
                        _conv_chunk(nc, ps, w_sb[0], xbv, ro, nr, co)
                        psv = ps[:, 0:nr * W].rearrange("p (r c) -> p r c", c=W)
                        nc.scalar.activation(
                            abv[:, 1 + ro:1 + ro + nr, co, 1:57], psv, AF.Sign,
                            bias=bn_sb[:, 2 + co:3 + co], scale=bn_sb[:, 0 + co:1 + co],
                        )

                # ---- conv2 -> bn2 + residual + clip -> store per chunk ----
                for co in range(2):
                    ost = ostp.tile([128, HW], dt.float32, tag="ost")
                    ostv = ost[:].rearrange("p (h w) -> p h w", w=W)
                    xinv = xiq[:, co].rearrange("p (h w) -> p h w", w=W)
                    for ro, nr in CH2:
                        fd = nr * W
                        pst = psp.tile([128, CHU], dt.float32, tag="ps")
                        _conv_chunk(nc, pst, w_sb[1], abv, ro, nr, co)
                        psv = pst[:, 0:fd].rearrange("p (r c) -> p r c", c=W)
                        tm = tmpp.tile([128, CHU], dt.float32, tag="tmp")
                        tmv = tm[:, 0:fd].rearrange("p (r c) -> p r c", c=W)
                        nc.scalar.activation(
                            tmv, psv, AF.Identity,
                            bias=bn_sb[:, 6 + co:7 + co], scale=bn_sb[:, 4 + co:5 + co],
                        )
                        ov = ostv[:, ro:ro + nr, :]
                        nc.vector.tensor_tensor(
                            ov, tmv, xinv[:, ro:ro + nr, :], ALU.add
                        )
                        nc.vector.tensor_scalar(ov, ov, 1.0, -1.0, ALU.min, ALU.max)
                        # last chunk's store rides the scalar queue: at the
                        # kernel tail the sync engine lags on earlier stores
                        st_eng = nc.scalar if ro == 54 else nc.sync
                        st_eng.dma_start(
                            out_d[n, co * 128:(co + 1) * 128, ro:ro + nr].rearrange(
                                "p h w -> p (h w)"),
                            ost[:, ro * W:(ro + nr) * W],
                        )

    nc.compile()
    return nc


def _get_nc():
    if "nc" not in _CACHE:
        _CACHE["nc"] = _build()
    return _CACHE["nc"]


def _prep_weights(w):
    # [co, cin, kh, kw] -> [cin_chunk 2, cin 128, tap 9, co 256], binarized fp8e4
    a = np.sign(w.astype(np.float32))
    a = a.transpose(1, 2, 3, 0).reshape(2, 128, 9, C)
    return np.ascontiguousarray(a.astype(ml_dtypes.float8_e4m3))


def _fold_bn(g, b, m, v):
    s = (g.astype(np.float32) / np.sqrt(v.astype(np.float32) + BN_EPS)).astype(np.float32)
    t = (b.astype(np.float32) - m.astype(np.float32) * s).astype(np.float32)
    return s.reshape(2, 128), t.reshape(2, 128)


def _prep_in_maps(x, w1, g1, b1, m1, v1, w2, g2, b2, m2, v2):
    w1b = _prep_weights(w1)
    w2b = _prep_weights(w2)
    s1, t1 = _fold_bn(g1, b1, m1, v1)
    s2, t2 = _fold_bn(g2, b2, m2, v2)
    bn = np.ascontiguousarray(np.stack([s1, t1, s2, t2]))  # [4, 2, 128]
    x = np.ascontiguousarray(x.astype(np.float32, copy=False))
    return [{
        "x": x[c * NPER:(c + 1) * NPER],
        "w1b": w1b, "w2b": w2b, "bn": bn,
    } for c in range(N_CORES)]


def kernel(x, w1, g1, b1, m1, v1, w2, g2, b2, m2, v2):
    nc = _get_nc()
    in_maps = _prep_in_maps(x, w1, g1, b1, m1, v1, w2, g2, b2, m2, v2)
    res = run_bass_kernel_spmd(nc, in_maps, list(range(N_CORES)))
    out = np.concatenate([res.results[c]["out"] for c in range(N_CORES)], axis=0)
    return out
